# revision 1
# baseline (speedup 1.0000x reference)
"""BayesianLinear TRN2 kernel: out = x @ (mu + (softplus(rho)+1e-8)*eps).T + bias.

Full shapes: x [4096, 4096], weight_* [4096(out), 4096(in)], bias_* [4096].
Sharding across 8 NeuronCores: 2 batch-halves x 4 out-groups.
  core c: batch rows [ (c//4)*2048 : ... ), out cols [ (c%4)*1024 : ... ).
Per core: out_shard [2048, 1024] = xT_h.T @ wT_g + bias_g.

Host-side prep is layout only (transpose/slice); all arithmetic (softplus,
weight sampling, matmul, bias) runs on device. Matmuls use float32r
(TF32-like, 11-bit mantissa, full PE rate); accumulation fp32 in PSUM.

K(=in_features) is processed in 8 phases of 512; partial products are
accumulated into an SBUF fp32 accumulator (PSUM only holds 8 banks).
"""
import numpy as np
from contextlib import ExitStack

import concourse.tile as tile
import concourse.mybir as mybir
from concourse import bacc

P = 128
IN_F = 4096           # contraction (in_features)
BATCH = 4096
OUT_F = 4096
B_CORE = 2048         # batch rows per core (2 halves)
O_CORE = 1024         # out cols per core (4 groups)
N_PHASES = 8
KC_PER_PHASE = (IN_F // P) // N_PHASES   # 4 k-chunks of 128 per phase
MG = 4                # m-groups of 512 batch per core
MS = 4                # m-subtiles of 128 per m-group
NH = O_CORE // 512    # 2 psum-width halves of out cols

F32 = mybir.dt.float32
F32R = mybir.dt.float32r
ACT = mybir.ActivationFunctionType
ALU = mybir.AluOpType

_CACHE = {}


def build_nc():
    nc = bacc.Bacc("TRN2", debug=False, num_devices=8)
    xt = nc.dram_tensor("xt", (IN_F, B_CORE), F32, kind="ExternalInput").ap()
    wtm = nc.dram_tensor("wtm", (IN_F, O_CORE), F32, kind="ExternalInput").ap()
    wtr = nc.dram_tensor("wtr", (IN_F, O_CORE), F32, kind="ExternalInput").ap()
    wte = nc.dram_tensor("wte", (IN_F, O_CORE), F32, kind="ExternalInput").ap()
    bm = nc.dram_tensor("bm", (O_CORE,), F32, kind="ExternalInput").ap()
    br = nc.dram_tensor("br", (O_CORE,), F32, kind="ExternalInput").ap()
    be = nc.dram_tensor("be", (O_CORE,), F32, kind="ExternalInput").ap()
    out = nc.dram_tensor("out", (B_CORE, O_CORE), F32, kind="ExternalOutput").ap()

    xt_t = xt.rearrange("(kc p) b -> kc p b", p=P)       # [32, 128, 2048]
    wm_t = wtm.rearrange("(kc p) o -> kc p o", p=P)      # [32, 128, 1024]
    wr_t = wtr.rearrange("(kc p) o -> kc p o", p=P)
    we_t = wte.rearrange("(kc p) o -> kc p o", p=P)
    out_t = out.rearrange("(mt p) o -> mt p o", p=P)     # [16, 128, 1024]

    with ExitStack() as ctx:
        tc = ctx.enter_context(tile.TileContext(nc))
        wpool = ctx.enter_context(tc.tile_pool(name="w", bufs=2))
        wstage = ctx.enter_context(tc.tile_pool(name="ws", bufs=2))
        xpool = ctx.enter_context(tc.tile_pool(name="x", bufs=2))
        accpool = ctx.enter_context(tc.tile_pool(name="acc", bufs=1))
        bpool = ctx.enter_context(tc.tile_pool(name="bias", bufs=1))
        pspool = ctx.enter_context(tc.tile_pool(name="ps", bufs=8, space="PSUM"))

        acc = accpool.tile([P, B_CORE // P, O_CORE], F32)   # 64KB/partition

        # ---- bias = bm + (softplus(br)+1e-8)*be, broadcast to all partitions
        bias_b = bpool.tile([P, O_CORE], F32)
        t_r = wstage.tile([P, O_CORE], F32, tag="rho")
        t_m = wstage.tile([P, O_CORE], F32, tag="mu")
        t_e = wstage.tile([P, O_CORE], F32, tag="eps")
        nc.sync.dma_start(t_r[:], br[None, :].to_broadcast((P, O_CORE)))
        nc.sync.dma_start(t_m[:], bm[None, :].to_broadcast((P, O_CORE)))
        nc.sync.dma_start(t_e[:], be[None, :].to_broadcast((P, O_CORE)))
        nc.scalar.activation(t_r[:], t_r[:], ACT.Exp)
        nc.scalar.activation(t_r[:], t_r[:], ACT.Ln, bias=1.0)
        nc.vector.scalar_tensor_tensor(t_r[:], t_r[:], 1e-8, t_e[:], ALU.add, ALU.mult)
        nc.vector.tensor_add(bias_b[:], t_r[:], t_m[:])

        # ---- main K phases
        for p in range(N_PHASES):
            # prep this phase's weight chunk: w = mu + (softplus(rho)+1e-8)*eps
            w_p = wpool.tile([P, KC_PER_PHASE, O_CORE], F32R, tag="wr")
            for kc in range(KC_PER_PHASE):
                k = p * KC_PER_PHASE + kc
                t_r = wstage.tile([P, O_CORE], F32, tag="rho")
                t_m = wstage.tile([P, O_CORE], F32, tag="mu")
                t_e = wstage.tile([P, O_CORE], F32, tag="eps")
                nc.sync.dma_start(t_r[:], wr_t[k])
                nc.sync.dma_start(t_m[:], wm_t[k])
                nc.sync.dma_start(t_e[:], we_t[k])
                nc.scalar.activation(t_r[:], t_r[:], ACT.Exp)
                nc.scalar.activation(t_r[:], t_r[:], ACT.Ln, bias=1.0)
                nc.vector.scalar_tensor_tensor(
                    t_r[:], t_r[:], 1e-8, t_e[:], ALU.add, ALU.mult
                )
                nc.vector.tensor_add(w_p[:, kc], t_r[:], t_m[:])

            for g in range(MG):
                xs = xpool.tile([P, KC_PER_PHASE, 512], F32, tag="xs")
                for kc in range(KC_PER_PHASE):
                    k = p * KC_PER_PHASE + kc
                    nc.sync.dma_start(
                        xs[:, kc], xt_t[k, :, g * 512:(g + 1) * 512]
                    )
                xr = xpool.tile([P, KC_PER_PHASE, 512], F32R, tag="xr")
                nc.gpsimd.tensor_copy(xr[:], xs[:])

                for ms in range(MS):
                    m = g * MS + ms
                    for n in range(NH):
                        ps = pspool.tile([P, 512], F32, tag="ps")
                        for kc in range(KC_PER_PHASE):
                            nc.tensor.matmul(
                                ps[:],
                                xr[:, kc, ms * P:(ms + 1) * P],
                                w_p[:, kc, n * 512:(n + 1) * 512],
                                start=(kc == 0),
                                stop=(kc == KC_PER_PHASE - 1),
                            )
                        a = acc[:, m, n * 512:(n + 1) * 512]
                        if p == 0:
                            nc.vector.tensor_add(a, ps[:], bias_b[:, n * 512:(n + 1) * 512])
                        else:
                            nc.vector.tensor_add(a, a, ps[:])
                    if p == N_PHASES - 1:
                        nc.sync.dma_start(out_t[m], acc[:, m, :])
    nc.compile()
    return nc


# ---------------------------------------------------------------------------
# host-side runner (PJRT under axon)
# ---------------------------------------------------------------------------

def _prepare_fn(nc, n_cores=8):
    import jax
    from jax.sharding import Mesh, PartitionSpec
    from jax.experimental.shard_map import shard_map
    from concourse.bass2jax import (
        _bass_exec_p, install_neuronx_cc_hook, partition_id_tensor,
    )

    install_neuronx_cc_hook()
    pname = nc.partition_id_tensor.name if nc.partition_id_tensor else None
    in_names, out_names, out_avals = [], [], []
    for alloc in nc.m.functions[0].allocations:
        if not isinstance(alloc, mybir.MemoryLocationSet):
            continue
        name = alloc.memorylocations[0].name
        if alloc.kind == "ExternalInput":
            if name != pname:
                in_names.append(name)
        elif alloc.kind == "ExternalOutput":
            out_names.append(name)
            out_avals.append(
                jax.core.ShapedArray(tuple(alloc.tensor_shape), mybir.dt.np(alloc.dtype))
            )

    all_in = list(in_names) + list(out_names) + ([pname] if pname else [])

    def _body(*args):
        ops = list(args)
        if pname:
            ops.append(partition_id_tensor())
        return tuple(
            _bass_exec_p.bind(
                *ops,
                out_avals=tuple(out_avals),
                in_names=tuple(all_in),
                out_names=tuple(out_names),
                lowering_input_output_aliases=(),
                sim_require_finite=True,
                sim_require_nnan=True,
                nc=nc,
            )
        )

    devices = jax.devices()[:n_cores]
    mesh = Mesh(np.asarray(devices), ("core",))
    nargs = len(in_names) + len(out_names)
    fn = jax.jit(
        shard_map(
            _body, mesh=mesh,
            in_specs=(PartitionSpec("core"),) * nargs,
            out_specs=(PartitionSpec("core"),) * len(out_names),
            check_rep=False,
        ),
        keep_unused=True,
    )
    return fn, mesh, in_names, out_names, out_avals


def get_compiled():
    if "fn" not in _CACHE:
        nc = build_nc()
        _CACHE["nc"] = nc
        _CACHE["fn"] = _prepare_fn(nc)
    return _CACHE["fn"]


def shard_inputs(x, weight_mu, weight_rho, bias_mu, bias_rho, weight_eps, bias_eps):
    """Returns in_maps (list of dicts, one per core). Layout-only transforms."""
    xT = np.ascontiguousarray(np.asarray(x).T)          # [in, batch]
    in_maps = []
    for c in range(8):
        h, g = divmod(c, 4)
        o0 = g * O_CORE
        in_maps.append({
            "xt": np.ascontiguousarray(xT[:, h * B_CORE:(h + 1) * B_CORE]),
            "wtm": np.ascontiguousarray(np.asarray(weight_mu)[o0:o0 + O_CORE, :].T),
            "wtr": np.ascontiguousarray(np.asarray(weight_rho)[o0:o0 + O_CORE, :].T),
            "wte": np.ascontiguousarray(np.asarray(weight_eps)[o0:o0 + O_CORE, :].T),
            "bm": np.asarray(bias_mu)[o0:o0 + O_CORE],
            "br": np.asarray(bias_rho)[o0:o0 + O_CORE],
            "be": np.asarray(bias_eps)[o0:o0 + O_CORE],
        })
    return in_maps


def run_device(in_maps):
    import jax
    from jax.sharding import NamedSharding, PartitionSpec

    fn, mesh, in_names, out_names, out_avals = get_compiled()
    sh = NamedSharding(mesh, PartitionSpec("core"))
    concat_in = [
        np.concatenate([np.asarray(in_maps[c][nm]) for c in range(8)], axis=0)
        for nm in in_names
    ]
    dev_in = [jax.device_put(a, sh) for a in concat_in]
    dev_z = [
        jax.device_put(np.zeros((8 * a.shape[0], *a.shape[1:]), a.dtype), sh)
        for a in out_avals
    ]
    out_arrs = fn(*dev_in, *dev_z)
    jax.block_until_ready(out_arrs)
    i_out = out_names.index("out")
    outs = np.asarray(out_arrs[i_out]).reshape(8, B_CORE, O_CORE)
    return outs, (fn, dev_in, dev_z)


def assemble(outs):
    full = np.empty((BATCH, OUT_F), dtype=np.float32)
    for c in range(8):
        h, g = divmod(c, 4)
        full[h * B_CORE:(h + 1) * B_CORE, g * O_CORE:(g + 1) * O_CORE] = outs[c]
    return full


def kernel(**inputs) -> np.ndarray:
    in_maps = shard_inputs(**inputs)
    outs, _ = run_device(in_maps)
    return assemble(outs)


if __name__ == "__main__":
    rng = np.random.default_rng(0)
    ins = {
        "x": rng.standard_normal((BATCH, IN_F), dtype=np.float32),
        "weight_mu": (rng.standard_normal((OUT_F, IN_F), dtype=np.float32)
                      * np.sqrt(2.0 / IN_F)).astype(np.float32),
        "weight_rho": rng.uniform(-5.5, -2.5, (OUT_F, IN_F)).astype(np.float32),
        "bias_mu": np.zeros(OUT_F, dtype=np.float32),
        "bias_rho": rng.uniform(-5.5, -2.5, OUT_F).astype(np.float32),
        "weight_eps": rng.standard_normal((OUT_F, IN_F), dtype=np.float32),
        "bias_eps": rng.standard_normal(OUT_F, dtype=np.float32),
    }
    got = kernel(**ins)
    w = ins["weight_mu"] + (np.log1p(np.exp(ins["weight_rho"].astype(np.float64))) + 1e-8) * ins["weight_eps"]
    b = ins["bias_mu"] + (np.log1p(np.exp(ins["bias_rho"].astype(np.float64))) + 1e-8) * ins["bias_eps"]
    ref = ins["x"].astype(np.float64) @ w.T + b
    rel = np.linalg.norm(got - ref) / np.linalg.norm(ref)
    print("L2 rel err vs fp64 numpy:", rel)
